# revision 2
# baseline (speedup 1.0000x reference)
"""Trainium2 Bass kernel for nn_AdaConv — Winograd F(2x2,3x3), v3.

Per sample: instance-norm -> per-sample 3x3 conv (512->512, reflect pad)
-> per-sample 1x1 conv + bias -> shared 3x3 conv + bias (reflect pad).

All input-dependent preprocessing happens on the host:
- pointwise composed into the 3x3 conv (W1' = dw @ pw)
- Winograd weight transform with the F(2,3) half-factors folded in
- instance-norm: per-channel a = rsqrt(var+eps) folded into u1's ci rows;
  the mean offset and pointwise bias ship as a per-sample beta vector
  added at transform point (1,1) during the PSUM drain
- x ships in the kernel's C-major, H-reflect-padded (66 rows),
  W-parity-split bf16 layout

Device: two back-to-back Winograd convs.  PE does 1024 [128x128]x[128,512]
bf16 matmuls; scalar drains PSUM (plus the strided y2 commit scatter);
vector does all transforms (W-build, H-pass, inverse stage1/stage2);
gpsimd only issues DMAs and the tiny y2 border copies.  Output leaves
C-major bf16 and is decoded on the host.
"""

import numpy as np

from concourse.bass_utils import run_bass_kernel_spmd

import concourse.bass as bass
import concourse.mybir as mybir
import concourse.tile as tile

F32 = mybir.dt.float32
BF16 = mybir.dt.bfloat16
AF = mybir.ActivationFunctionType

H = W = 64
C = 512
NK = 4
EPS = 1e-5
NSTRIP = 2          # tile-row strips of 16 (32 image rows each)
THS = 16            # tile rows per strip
TPS = THS * 32      # tiles per strip (512)


def _hpass(nc, V, T, u, eng=None):
    """H-dim transform point u: T [128, NK, 34, 32] -> V [128, NK, THS, 32]."""
    e = eng or nc.vector
    if u == 0:
        e.tensor_sub(out=V, in0=T[:, :, 0:31:2, :], in1=T[:, :, 2:33:2, :])
    elif u == 1:
        e.tensor_add(out=V, in0=T[:, :, 1:32:2, :], in1=T[:, :, 2:33:2, :])
    elif u == 2:
        e.tensor_sub(out=V, in0=T[:, :, 2:33:2, :], in1=T[:, :, 1:32:2, :])
    else:
        e.tensor_sub(out=V, in0=T[:, :, 1:32:2, :], in1=T[:, :, 3:34:2, :])


def _wbuild(nc, T, src, w):
    """W-dim transform point w from parity-split rows src [128, NK, 34, 2, 32]
    into T [128, NK, 34, 32]."""
    p0 = src[:, :, :, 0, :]
    p1 = src[:, :, :, 1, :]
    if w == 0:
        nc.vector.memset(T[:, :, :, 0:1], 0.0)
        nc.vector.tensor_sub(out=T[:, :, :, 1:32],
                             in0=p1[:, :, :, 0:31], in1=p1[:, :, :, 1:32])
    elif w == 1:
        nc.vector.tensor_add(out=T, in0=p0, in1=p1)
    elif w == 2:
        nc.vector.tensor_sub(out=T, in0=p1, in1=p0)
    else:
        nc.vector.memset(T[:, :, :, 31:32], 0.0)
        nc.vector.tensor_sub(out=T[:, :, :, 0:31],
                             in0=p0[:, :, :, 0:31], in1=p0[:, :, :, 1:32])


def build():
    nc = bass.Bass()
    x_d = nc.declare_dram_parameter("x", [C, 66 * 64], BF16, isOutput=False)
    u1_d = nc.declare_dram_parameter("u1", [C, 16 * C], BF16, isOutput=False)
    u3_d = nc.declare_dram_parameter("u3", [C, 16 * C], BF16, isOutput=False)
    pb_d = nc.declare_dram_parameter("pb", [C, 1], F32, isOutput=False)
    cb_d = nc.declare_dram_parameter("cb", [C, 1], F32, isOutput=False)
    # out[c, s, ab, t]: h = 32s + 2*(t//32) + ab//2, w = 2*(t%32) + ab%2
    out_d = nc.declare_dram_parameter("out", [C, NSTRIP, 4, TPS], BF16,
                                      isOutput=True)

    with tile.TileContext(nc) as tc:
        with (
            tc.tile_pool(name="main", bufs=1) as mp,
            tc.tile_pool(name="psMM", bufs=8, space="PSUM") as psMM,
        ):
            pb_sb = mp.tile([128, NK, 1], F32, tag="pb")
            cb_sb = mp.tile([128, NK, 1], F32, tag="cb")

            y2b = mp.tile([128, NK, 66, 2, 32], BF16, tag="y2b")
            xb = mp.tile([128, NK, 66, 2, 32], BF16, tag="xb")
            # x: strip-1 rows (32..65) first -- they are processed first
            dmae = [nc.sync, nc.scalar, nc.gpsimd]
            for k in range(NK):
                dmae[k % 3].dma_start(out=xb[:, k, 32:66, :, :],
                                      in_=x_d[k * 128:(k + 1) * 128,
                                              32 * 64:66 * 64])
            # interleave-order schedule: conv1 strips [1,0], conv3 strips [1,0]
            sched = [(1, s, w) for s in (1, 0) for w in range(4)] + \
                    [(3, s, w) for s in (1, 0) for w in range(4)]
            wsrc = {1: u1_d, 3: u3_d}

            def emit_chunk(i):
                cv, s, w = sched[i]
                Uw = mp.tile([128, NK, 4, C], BF16, tag="Uw", bufs=3,
                             name=f"Uw{cv}_{s}_{w}")
                for k in range(NK):
                    (nc.sync, nc.gpsimd)[(i + k) % 2].dma_start(
                        out=Uw[:, k, :, :],
                        in_=wsrc[cv][k * 128:(k + 1) * 128,
                                     w * 2048:(w + 1) * 2048])
                return Uw

            chunks = [emit_chunk(0), emit_chunk(1)]
            # x strip-0 rows after the first weight chunks (needed later)
            for k in range(NK):
                dmae[k % 3].dma_start(out=xb[:, k, 0:32, :, :],
                                      in_=x_d[k * 128:(k + 1) * 128, 0:32 * 64])
            for k in range(NK):
                nc.gpsimd.dma_start(out=pb_sb[:, k, :],
                                    in_=pb_d[k * 128:(k + 1) * 128, :])
                nc.gpsimd.dma_start(out=cb_sb[:, k, :],
                                    in_=cb_d[k * 128:(k + 1) * 128, :])

            def commit_y2(s, yst):
                # strided scatter into y2b rows on the scalar engine
                r0 = 1 + 32 * s
                for ab in (2, 3, 0, 1):
                    a, bb = ab // 2, ab % 2
                    nc.scalar.activation(
                        out=y2b[:, :, r0 + a:r0 + a + 31:2, bb, :],
                        in_=yst[:, :, ab, :], func=AF.Copy)
                # reflect border rows as soon as their source row exists
                if s == 1:
                    nc.scalar.activation(out=y2b[:, :, 65, :, :],
                                         in_=y2b[:, :, 63, :, :], func=AF.Copy)
                else:
                    nc.scalar.activation(out=y2b[:, :, 0, :, :],
                                         in_=y2b[:, :, 2, :, :], func=AF.Copy)

            def emit_out(s, yst):
                for k in range(NK):
                    (nc.sync, nc.gpsimd)[k % 2].dma_start(
                        out=out_d[k * 128:(k + 1) * 128, s, :, :],
                        in_=yst[:, k, :, :])

            def flush(pend):
                cv, s, w, Mq, yst = pend
                s0a = yst[:, :, 0, :]
                s1a = yst[:, :, 1, :]
                s2a = yst[:, :, 2, :]
                s3a = yst[:, :, 3, :]
                y0 = yst[:, :, 0:3:2, :]
                y1 = yst[:, :, 1:4:2, :]
                if w == 0:
                    # stage1 writes y0 slots (a0->slot0, a1->slot2) directly
                    nc.vector.tensor_add(out=s0a, in0=Mq[:, :, 0, :], in1=Mq[:, :, 1, :])
                    nc.vector.tensor_add(out=s0a, in0=s0a, in1=Mq[:, :, 2, :])
                    nc.vector.tensor_sub(out=s2a, in0=Mq[:, :, 1, :], in1=Mq[:, :, 2, :])
                    nc.vector.tensor_sub(out=s2a, in0=s2a, in1=Mq[:, :, 3, :])
                elif w == 1:
                    # stage1 into y1 slots, then y0 += y1
                    nc.vector.tensor_add(out=s1a, in0=Mq[:, :, 0, :], in1=Mq[:, :, 1, :])
                    nc.vector.tensor_add(out=s1a, in0=s1a, in1=Mq[:, :, 2, :])
                    nc.vector.tensor_sub(out=s3a, in0=Mq[:, :, 1, :], in1=Mq[:, :, 2, :])
                    nc.vector.tensor_sub(out=s3a, in0=s3a, in1=Mq[:, :, 3, :])
                    nc.vector.tensor_add(out=y0, in0=y0, in1=y1)
                else:
                    tq = mp.tile([128, NK, 2, TPS], BF16, tag="tq", bufs=1,
                                 name=f"tq{cv}_{s}_{w}")
                    t0 = tq[:, :, 0, :]
                    t1 = tq[:, :, 1, :]
                    nc.vector.tensor_add(out=t0, in0=Mq[:, :, 0, :], in1=Mq[:, :, 1, :])
                    nc.vector.tensor_add(out=t0, in0=t0, in1=Mq[:, :, 2, :])
                    nc.vector.tensor_sub(out=t1, in0=Mq[:, :, 1, :], in1=Mq[:, :, 2, :])
                    nc.vector.tensor_sub(out=t1, in0=t1, in1=Mq[:, :, 3, :])
                    t = tq[:, :, :, :]
                    if w == 2:
                        nc.vector.tensor_add(out=y0, in0=y0, in1=t)
                        nc.vector.tensor_sub(out=y1, in0=y1, in1=t)
                    elif cv == 1:
                        nc.vector.tensor_sub(out=y1, in0=y1, in1=t)
                        commit_y2(s, yst)
                    else:
                        # per-k: finish y1 and ship that k immediately
                        for k in range(NK):
                            nc.vector.tensor_sub(out=y1[:, k, :, :],
                                                 in0=y1[:, k, :, :],
                                                 in1=t[:, k, :, :])
                            (nc.sync, nc.scalar)[k % 2].dma_start(
                                out=out_d[k * 128:(k + 1) * 128, s, :, :],
                                in_=yst[:, k, :, :])

            srcs = {1: xb, 3: y2b}
            bias5 = {1: pb_sb, 3: cb_sb}
            pend = None
            yst_by = {}
            for i, (cv, s, w) in enumerate(sched):
                if i + 2 < len(sched):
                    chunks.append(emit_chunk(i + 2))
                Uw = chunks[i]
                src = srcs[cv]
                r0 = 32 * s
                if w == 0:
                    yst_by[(cv, s)] = mp.tile([128, NK, 4, TPS], BF16,
                                              tag="yst", bufs=2,
                                              name=f"yst{cv}_{s}")
                yst = yst_by[(cv, s)]
                T = mp.tile([128, NK, 34, 32], BF16, tag="T", bufs=1,
                            name=f"T{cv}_{s}_{w}")
                _wbuild(nc, T, src[:, :, r0:r0 + 34, :, :], w)
                Mq = mp.tile([128, NK, 4, TPS], BF16, tag="Mq", bufs=2,
                             name=f"Mq{cv}_{s}_{w}")
                for u in range(4):
                    ptid = w * 4 + u
                    V = mp.tile([128, NK, THS, 32], BF16, tag="V", bufs=3,
                                name=f"V{cv}_{s}_{ptid}")
                    _hpass(nc, V, T, u)
                    for co in range(NK):
                        ps = psMM.tile([128, TPS], F32, tag="mm",
                                       name=f"ps{cv}_{s}_{ptid}_{co}")
                        for ci in range(NK):
                            nc.tensor.matmul(
                                ps[:, :],
                                Uw[:, ci, u, co * 128:(co + 1) * 128],
                                V[:, ci, :, :],
                                start=(ci == 0), stop=(ci == NK - 1))
                        if ptid == 5:
                            nc.scalar.activation(
                                out=Mq[:, co, u, :], in_=ps,
                                func=AF.Identity, bias=bias5[cv][:, co, :])
                        else:
                            nc.scalar.activation(
                                out=Mq[:, co, u, :], in_=ps, func=AF.Copy)
                if pend is not None:
                    flush(pend)
                pend = (cv, s, w, Mq, yst)
                if cv == 1 and s == 0 and w == 3:
                    # conv boundary: eager flush (conv3 strip1 needs row 32)
                    flush(pend)
                    pend = None
            if pend is not None:
                flush(pend)

    return nc


# ---- walrus single-wait workaround (same as baseline) ----
import concourse.tile as tile_mod
from concourse.vector_clock import ScopedClock

MAX_WAITS = 1
_counter = [0]


def _fresh_name(base):
    _counter[0] += 1
    return f"{base}-wsplit-{_counter[0]}"


_orig_add_instruction = tile_mod.TileContext._add_instruction


def _patched_add_instruction(self, inst):
    si = getattr(inst, "sync_info", None)
    if si is not None and si.on_wait is not None and len(si.on_wait) > MAX_WAITS:
        waits = list(si.on_wait)
        for w in waits[:-MAX_WAITS]:
            nop = mybir.InstNoOp(
                name=_fresh_name(inst.name),
                sync_info=mybir.SyncInfo(on_wait=[w], on_update=[]),
                bass_nofuse=True,
                engine=inst.engine,
            )
            _orig_add_instruction(self, nop)
        inst.sync_info = mybir.SyncInfo(
            on_wait=waits[-MAX_WAITS:], on_update=list(si.on_update)
        )
    _orig_add_instruction(self, inst)


def _split_tail_waits(nc, raw):
    si = raw.sync_info
    waits = list(si.on_wait) if si is not None else []
    if len(waits) <= MAX_WAITS:
        return
    updates = list(si.on_update) if si is not None else []
    raw.sync_info = mybir.SyncInfo(on_wait=waits[:MAX_WAITS], on_update=updates)
    for i in range(MAX_WAITS, len(waits), MAX_WAITS):
        extra = nc.sync.drain()
        extra.ins.sync_info = mybir.SyncInfo(
            on_wait=waits[i : i + MAX_WAITS], on_update=[]
        )


def _patched_drain_and_barrier(self, tick_clock, wait_clock):
    nc = self.nc
    drain_inst = nc.sync.drain()
    wait_clock.add_sem_waits(
        drain_inst.ins, ScopedClock({None: tick_clock.global_clock})
    )
    _split_tail_waits(nc, drain_inst.ins)

    nc.all_engine_barrier()
    assert self.sems is not None
    popped = nc._tile_sem_poison_stack.pop()
    assert popped is self._sem_poison
    nc.clear_and_free_semaphores(list(self.sems.allocated().values()))
    nc.all_engine_barrier()


def install():
    tile_mod.TileContext._add_instruction = _patched_add_instruction
    tile_mod.TileContext._drain_and_barrier = _patched_drain_and_barrier


_cached_nc = None


def _get_nc():
    global _cached_nc
    if _cached_nc is None:
        install()
        _cached_nc = build()
    return _cached_nc


def _wino_weights_folded(w, rowscale=None):
    """Host F(2,3) weight transform with the inverse-transform half-factors
    folded in (standard G with 1/2 rows): w [3,3,C,Co] -> [C, 16*Co] bf16.
    rowscale [C]: optional per-input-channel scale (instance-norm a)."""
    import ml_dtypes
    G = np.array([[1, 0, 0], [.5, .5, .5], [.5, -.5, .5], [0, 0, 1]], np.float32)
    U = np.einsum("ui,ijco,vj->uvco", G, np.asarray(w, np.float32), G,
                  optimize=True)
    if rowscale is not None:
        U = U * rowscale[None, None, :, None]
    U = U.transpose(2, 1, 0, 3).reshape(w.shape[2], 16 * w.shape[3])
    return np.ascontiguousarray(U.astype(ml_dtypes.bfloat16))


def _pack_x(xi):
    """x [H,W,C] f32 -> [C, 66*64] bf16 in C-major, H-reflect-padded,
    W-parity-split layout: out[c, r, p, j] = x[r-1, 2j+p, c]."""
    import ml_dtypes
    xc = np.asarray(xi, np.float32).transpose(2, 0, 1)          # [C,H,W]
    core = xc.reshape(C, H, 32, 2).transpose(0, 1, 3, 2)        # [C,H,p,j]
    arr = np.empty((C, 66, 2, 32), np.float32)
    arr[:, 1:65] = core
    arr[:, 0] = core[:, 1]
    arr[:, 65] = core[:, 62]
    return np.ascontiguousarray(
        arr.reshape(C, 66 * 64).astype(ml_dtypes.bfloat16))


def make_in_maps(x, depthwise_kernels, pointwise_kernels, biases, conv_w, conv_b):
    B = 8
    u3 = _wino_weights_folded(np.asarray(conv_w, np.float32))
    cbr = np.ascontiguousarray(np.asarray(conv_b, np.float32).reshape(C, 1))
    xs = np.asarray(x, np.float32)
    dw = np.asarray(depthwise_kernels, np.float32)
    pw = np.asarray(pointwise_kernels, np.float32)
    bs = np.asarray(biases, np.float32)
    in_maps = []
    for i in range(B):
        w1 = (dw[i].reshape(9 * C, C) @ pw[i, 0, 0]).reshape(3, 3, C, C)
        mean = xs[i].mean(axis=(0, 1))
        var = xs[i].var(axis=(0, 1))
        a = 1.0 / np.sqrt(var + EPS)
        # beta: constant (-a*mean) pushed through the conv, plus pw bias
        beta = np.einsum("c,ijco->o", -a * mean, w1) + bs[i]
        in_maps.append({
            "x": _pack_x(xs[i]),
            "u1": _wino_weights_folded(w1, rowscale=a),
            "u3": u3,
            "pb": np.ascontiguousarray(beta.reshape(C, 1)),
            "cb": cbr,
        })
    return in_maps


def _decode_out(o):
    # o [C, 2, 4, TPS] bf16 -> [H, W, C] f32
    a = np.asarray(o, np.float32).reshape(C, 2, 2, 2, THS, 32)
    # dims [c, s, a, b, th, tw] -> h = 32s+2th+a, w = 2tw+b
    return np.ascontiguousarray(
        a.transpose(1, 4, 2, 5, 3, 0).reshape(H, W, C))


def kernel(x, depthwise_kernels, pointwise_kernels, biases, conv_w, conv_b):
    B = 8
    nc = _get_nc()
    in_maps = make_in_maps(x, depthwise_kernels, pointwise_kernels, biases,
                           conv_w, conv_b)
    res = run_bass_kernel_spmd(nc, in_maps, core_ids=list(range(B)))
    return np.stack([_decode_out(res.results[i]["out"]) for i in range(B)])


# revision 3
# speedup vs baseline: 1.0064x; 1.0064x over previous
"""Trainium2 Bass kernel for nn_AdaConv — Winograd F(2x2,3x3), v3.

Per sample: instance-norm -> per-sample 3x3 conv (512->512, reflect pad)
-> per-sample 1x1 conv + bias -> shared 3x3 conv + bias (reflect pad).

All input-dependent preprocessing happens on the host:
- pointwise composed into the 3x3 conv (W1' = dw @ pw)
- Winograd weight transform with the F(2,3) half-factors folded in
- instance-norm: per-channel a = rsqrt(var+eps) folded into u1's ci rows;
  the mean offset and pointwise bias ship as a per-sample beta vector
  added at transform point (1,1) during the PSUM drain
- x ships in the kernel's C-major, H-reflect-padded (66 rows),
  W-parity-split bf16 layout

Device: two back-to-back Winograd convs.  PE does 1024 [128x128]x[128,512]
bf16 matmuls; scalar drains PSUM (plus the strided y2 commit scatter);
vector does all transforms (W-build, H-pass, inverse stage1/stage2);
gpsimd only issues DMAs and the tiny y2 border copies.  Output leaves
C-major bf16 and is decoded on the host.
"""

import numpy as np

from concourse.bass_utils import run_bass_kernel_spmd

import concourse.bass as bass
import concourse.mybir as mybir
import concourse.tile as tile

F32 = mybir.dt.float32
BF16 = mybir.dt.bfloat16
AF = mybir.ActivationFunctionType

H = W = 64
C = 512
NK = 4
EPS = 1e-5
NSTRIP = 2          # tile-row strips of 16 (32 image rows each)
THS = 16            # tile rows per strip
TPS = THS * 32      # tiles per strip (512)


def _hpass(nc, V, T, u, eng=None):
    """H-dim transform point u: T [128, NK, 34, 32] -> V [128, NK, THS, 32]."""
    e = eng or nc.vector
    if u == 0:
        e.tensor_sub(out=V, in0=T[:, :, 0:31:2, :], in1=T[:, :, 2:33:2, :])
    elif u == 1:
        e.tensor_add(out=V, in0=T[:, :, 1:32:2, :], in1=T[:, :, 2:33:2, :])
    elif u == 2:
        e.tensor_sub(out=V, in0=T[:, :, 2:33:2, :], in1=T[:, :, 1:32:2, :])
    else:
        e.tensor_sub(out=V, in0=T[:, :, 1:32:2, :], in1=T[:, :, 3:34:2, :])


def _wbuild(nc, T, src, w):
    """W-dim transform point w from parity-split rows src [128, NK, 34, 2, 32]
    into T [128, NK, 34, 32]."""
    p0 = src[:, :, :, 0, :]
    p1 = src[:, :, :, 1, :]
    if w == 0:
        nc.vector.memset(T[:, :, :, 0:1], 0.0)
        nc.vector.tensor_sub(out=T[:, :, :, 1:32],
                             in0=p1[:, :, :, 0:31], in1=p1[:, :, :, 1:32])
    elif w == 1:
        nc.vector.tensor_add(out=T, in0=p0, in1=p1)
    elif w == 2:
        nc.vector.tensor_sub(out=T, in0=p1, in1=p0)
    else:
        nc.vector.memset(T[:, :, :, 31:32], 0.0)
        nc.vector.tensor_sub(out=T[:, :, :, 0:31],
                             in0=p0[:, :, :, 0:31], in1=p0[:, :, :, 1:32])


def build():
    nc = bass.Bass()
    x_d = nc.declare_dram_parameter("x", [C, 66 * 64], BF16, isOutput=False)
    u1_d = nc.declare_dram_parameter("u1", [C, 16 * C], BF16, isOutput=False)
    u3_d = nc.declare_dram_parameter("u3", [C, 16 * C], BF16, isOutput=False)
    pb_d = nc.declare_dram_parameter("pb", [C, 1], F32, isOutput=False)
    cb_d = nc.declare_dram_parameter("cb", [C, 1], F32, isOutput=False)
    # out[c, s, ab, t]: h = 32s + 2*(t//32) + ab//2, w = 2*(t%32) + ab%2
    out_d = nc.declare_dram_parameter("out", [C, NSTRIP, 4, TPS], BF16,
                                      isOutput=True)

    with tile.TileContext(nc) as tc:
        with (
            tc.tile_pool(name="main", bufs=1) as mp,
            tc.tile_pool(name="psMM", bufs=8, space="PSUM") as psMM,
        ):
            pb_sb = mp.tile([128, NK, 1], F32, tag="pb")
            cb_sb = mp.tile([128, NK, 1], F32, tag="cb")

            y2b = mp.tile([128, NK, 66, 2, 32], BF16, tag="y2b")
            xb = mp.tile([128, NK, 66, 2, 32], BF16, tag="xb")
            # x: strip-1 rows (32..65) first -- they are processed first
            dmae = [nc.sync, nc.scalar, nc.gpsimd]
            for k in range(NK):
                dmae[k % 3].dma_start(out=xb[:, k, 32:66, :, :],
                                      in_=x_d[k * 128:(k + 1) * 128,
                                              32 * 64:66 * 64])
            # interleave-order schedule: conv1 strips [1,0], conv3 strips [1,0]
            sched = [(1, s, w) for s in (1, 0) for w in range(4)] + \
                    [(3, s, w) for s in (1, 0) for w in range(4)]
            wsrc = {1: u1_d, 3: u3_d}

            def emit_chunk(i):
                cv, s, w = sched[i]
                Uw = mp.tile([128, NK, 4, C], BF16, tag="Uw", bufs=3,
                             name=f"Uw{cv}_{s}_{w}")
                for k in range(NK):
                    (nc.sync, nc.gpsimd)[(i + k) % 2].dma_start(
                        out=Uw[:, k, :, :],
                        in_=wsrc[cv][k * 128:(k + 1) * 128,
                                     w * 2048:(w + 1) * 2048])
                return Uw

            chunks = [emit_chunk(0), emit_chunk(1)]
            # x strip-0 rows after the first weight chunks (needed later)
            for k in range(NK):
                dmae[k % 3].dma_start(out=xb[:, k, 0:32, :, :],
                                      in_=x_d[k * 128:(k + 1) * 128, 0:32 * 64])
            for k in range(NK):
                nc.gpsimd.dma_start(out=pb_sb[:, k, :],
                                    in_=pb_d[k * 128:(k + 1) * 128, :])
                nc.gpsimd.dma_start(out=cb_sb[:, k, :],
                                    in_=cb_d[k * 128:(k + 1) * 128, :])

            def commit_y2(s, yst):
                # strided scatter into y2b rows on the scalar engine
                r0 = 1 + 32 * s
                for ab in (3, 2, 1, 0):
                    a, bb = ab // 2, ab % 2
                    nc.scalar.activation(
                        out=y2b[:, :, r0 + a:r0 + a + 31:2, bb, :],
                        in_=yst[:, :, ab, :], func=AF.Copy)
                # reflect border rows as soon as their source row exists
                if s == 1:
                    nc.scalar.activation(out=y2b[:, :, 65, :, :],
                                         in_=y2b[:, :, 63, :, :], func=AF.Copy)
                else:
                    nc.scalar.activation(out=y2b[:, :, 0, :, :],
                                         in_=y2b[:, :, 2, :, :], func=AF.Copy)

            def emit_out(s, yst):
                for k in range(NK):
                    (nc.sync, nc.gpsimd)[k % 2].dma_start(
                        out=out_d[k * 128:(k + 1) * 128, s, :, :],
                        in_=yst[:, k, :, :])

            def flush(pend):
                cv, s, w, Mq, yst = pend
                s0a = yst[:, :, 0, :]
                s1a = yst[:, :, 1, :]
                s2a = yst[:, :, 2, :]
                s3a = yst[:, :, 3, :]
                y0 = yst[:, :, 0:3:2, :]
                y1 = yst[:, :, 1:4:2, :]
                if w == 0:
                    # stage1 writes y0 slots (a0->slot0, a1->slot2) directly
                    nc.vector.tensor_add(out=s0a, in0=Mq[:, :, 0, :], in1=Mq[:, :, 1, :])
                    nc.vector.tensor_add(out=s0a, in0=s0a, in1=Mq[:, :, 2, :])
                    nc.vector.tensor_sub(out=s2a, in0=Mq[:, :, 1, :], in1=Mq[:, :, 2, :])
                    nc.vector.tensor_sub(out=s2a, in0=s2a, in1=Mq[:, :, 3, :])
                elif w == 1:
                    # stage1 into y1 slots, then y0 += y1
                    nc.vector.tensor_add(out=s1a, in0=Mq[:, :, 0, :], in1=Mq[:, :, 1, :])
                    nc.vector.tensor_add(out=s1a, in0=s1a, in1=Mq[:, :, 2, :])
                    nc.vector.tensor_sub(out=s3a, in0=Mq[:, :, 1, :], in1=Mq[:, :, 2, :])
                    nc.vector.tensor_sub(out=s3a, in0=s3a, in1=Mq[:, :, 3, :])
                    nc.vector.tensor_add(out=y0, in0=y0, in1=y1)
                else:
                    tq = mp.tile([128, NK, 2, TPS], BF16, tag="tq", bufs=1,
                                 name=f"tq{cv}_{s}_{w}")
                    t0 = tq[:, :, 0, :]
                    t1 = tq[:, :, 1, :]
                    # t1 first: at w==3 it alone gates the slot-3 commit
                    nc.vector.tensor_sub(out=t1, in0=Mq[:, :, 1, :], in1=Mq[:, :, 2, :])
                    nc.vector.tensor_sub(out=t1, in0=t1, in1=Mq[:, :, 3, :])
                    nc.vector.tensor_add(out=t0, in0=Mq[:, :, 0, :], in1=Mq[:, :, 1, :])
                    nc.vector.tensor_add(out=t0, in0=t0, in1=Mq[:, :, 2, :])
                    t = tq[:, :, :, :]
                    if w == 2:
                        nc.vector.tensor_add(out=y0, in0=y0, in1=t)
                        nc.vector.tensor_sub(out=y1, in0=y1, in1=t)
                    elif cv == 1:
                        nc.vector.tensor_sub(out=y1[:, :, 1, :],
                                             in0=y1[:, :, 1, :],
                                             in1=t[:, :, 1, :])
                        nc.vector.tensor_sub(out=y1[:, :, 0, :],
                                             in0=y1[:, :, 0, :],
                                             in1=t[:, :, 0, :])
                        commit_y2(s, yst)
                    else:
                        # per-k: finish y1 and ship that k immediately
                        for k in range(NK):
                            nc.vector.tensor_sub(out=y1[:, k, :, :],
                                                 in0=y1[:, k, :, :],
                                                 in1=t[:, k, :, :])
                            (nc.sync, nc.scalar)[k % 2].dma_start(
                                out=out_d[k * 128:(k + 1) * 128, s, :, :],
                                in_=yst[:, k, :, :])

            srcs = {1: xb, 3: y2b}
            bias5 = {1: pb_sb, 3: cb_sb}
            pend = None
            yst_by = {}
            for i, (cv, s, w) in enumerate(sched):
                if i + 2 < len(sched):
                    chunks.append(emit_chunk(i + 2))
                Uw = chunks[i]
                src = srcs[cv]
                r0 = 32 * s
                if w == 0:
                    yst_by[(cv, s)] = mp.tile([128, NK, 4, TPS], BF16,
                                              tag="yst", bufs=2,
                                              name=f"yst{cv}_{s}")
                yst = yst_by[(cv, s)]
                T = mp.tile([128, NK, 34, 32], BF16, tag="T", bufs=1,
                            name=f"T{cv}_{s}_{w}")
                _wbuild(nc, T, src[:, :, r0:r0 + 34, :, :], w)
                Mq = mp.tile([128, NK, 4, TPS], BF16, tag="Mq", bufs=2,
                             name=f"Mq{cv}_{s}_{w}")
                for u in range(4):
                    ptid = w * 4 + u
                    V = mp.tile([128, NK, THS, 32], BF16, tag="V", bufs=3,
                                name=f"V{cv}_{s}_{ptid}")
                    _hpass(nc, V, T, u)
                    for co in range(NK):
                        ps = psMM.tile([128, TPS], F32, tag="mm",
                                       name=f"ps{cv}_{s}_{ptid}_{co}")
                        for ci in range(NK):
                            nc.tensor.matmul(
                                ps[:, :],
                                Uw[:, ci, u, co * 128:(co + 1) * 128],
                                V[:, ci, :, :],
                                start=(ci == 0), stop=(ci == NK - 1))
                        if ptid == 5:
                            nc.scalar.activation(
                                out=Mq[:, co, u, :], in_=ps,
                                func=AF.Identity, bias=bias5[cv][:, co, :])
                        else:
                            nc.scalar.activation(
                                out=Mq[:, co, u, :], in_=ps, func=AF.Copy)
                if pend is not None:
                    flush(pend)
                pend = (cv, s, w, Mq, yst)
                if cv == 1 and s == 0 and w == 3:
                    # conv boundary: eager flush (conv3 strip1 needs row 32)
                    flush(pend)
                    pend = None
            if pend is not None:
                flush(pend)

    return nc


# ---- walrus single-wait workaround (same as baseline) ----
import concourse.tile as tile_mod
from concourse.vector_clock import ScopedClock

MAX_WAITS = 1
_counter = [0]


def _fresh_name(base):
    _counter[0] += 1
    return f"{base}-wsplit-{_counter[0]}"


_orig_add_instruction = tile_mod.TileContext._add_instruction


def _patched_add_instruction(self, inst):
    si = getattr(inst, "sync_info", None)
    if si is not None and si.on_wait is not None and len(si.on_wait) > MAX_WAITS:
        waits = list(si.on_wait)
        for w in waits[:-MAX_WAITS]:
            nop = mybir.InstNoOp(
                name=_fresh_name(inst.name),
                sync_info=mybir.SyncInfo(on_wait=[w], on_update=[]),
                bass_nofuse=True,
                engine=inst.engine,
            )
            _orig_add_instruction(self, nop)
        inst.sync_info = mybir.SyncInfo(
            on_wait=waits[-MAX_WAITS:], on_update=list(si.on_update)
        )
    _orig_add_instruction(self, inst)


def _split_tail_waits(nc, raw):
    si = raw.sync_info
    waits = list(si.on_wait) if si is not None else []
    if len(waits) <= MAX_WAITS:
        return
    updates = list(si.on_update) if si is not None else []
    raw.sync_info = mybir.SyncInfo(on_wait=waits[:MAX_WAITS], on_update=updates)
    for i in range(MAX_WAITS, len(waits), MAX_WAITS):
        extra = nc.sync.drain()
        extra.ins.sync_info = mybir.SyncInfo(
            on_wait=waits[i : i + MAX_WAITS], on_update=[]
        )


def _patched_drain_and_barrier(self, tick_clock, wait_clock):
    nc = self.nc
    drain_inst = nc.sync.drain()
    wait_clock.add_sem_waits(
        drain_inst.ins, ScopedClock({None: tick_clock.global_clock})
    )
    _split_tail_waits(nc, drain_inst.ins)

    nc.all_engine_barrier()
    assert self.sems is not None
    popped = nc._tile_sem_poison_stack.pop()
    assert popped is self._sem_poison
    nc.clear_and_free_semaphores(list(self.sems.allocated().values()))
    nc.all_engine_barrier()


def install():
    tile_mod.TileContext._add_instruction = _patched_add_instruction
    tile_mod.TileContext._drain_and_barrier = _patched_drain_and_barrier


_cached_nc = None


def _get_nc():
    global _cached_nc
    if _cached_nc is None:
        install()
        _cached_nc = build()
    return _cached_nc


def _wino_weights_folded(w, rowscale=None):
    """Host F(2,3) weight transform with the inverse-transform half-factors
    folded in (standard G with 1/2 rows): w [3,3,C,Co] -> [C, 16*Co] bf16.
    rowscale [C]: optional per-input-channel scale (instance-norm a)."""
    import ml_dtypes
    G = np.array([[1, 0, 0], [.5, .5, .5], [.5, -.5, .5], [0, 0, 1]], np.float32)
    U = np.einsum("ui,ijco,vj->uvco", G, np.asarray(w, np.float32), G,
                  optimize=True)
    if rowscale is not None:
        U = U * rowscale[None, None, :, None]
    U = U.transpose(2, 1, 0, 3).reshape(w.shape[2], 16 * w.shape[3])
    return np.ascontiguousarray(U.astype(ml_dtypes.bfloat16))


def _pack_x(xi):
    """x [H,W,C] f32 -> [C, 66*64] bf16 in C-major, H-reflect-padded,
    W-parity-split layout: out[c, r, p, j] = x[r-1, 2j+p, c]."""
    import ml_dtypes
    xc = np.asarray(xi, np.float32).transpose(2, 0, 1)          # [C,H,W]
    core = xc.reshape(C, H, 32, 2).transpose(0, 1, 3, 2)        # [C,H,p,j]
    arr = np.empty((C, 66, 2, 32), np.float32)
    arr[:, 1:65] = core
    arr[:, 0] = core[:, 1]
    arr[:, 65] = core[:, 62]
    return np.ascontiguousarray(
        arr.reshape(C, 66 * 64).astype(ml_dtypes.bfloat16))


def make_in_maps(x, depthwise_kernels, pointwise_kernels, biases, conv_w, conv_b):
    B = 8
    u3 = _wino_weights_folded(np.asarray(conv_w, np.float32))
    cbr = np.ascontiguousarray(np.asarray(conv_b, np.float32).reshape(C, 1))
    xs = np.asarray(x, np.float32)
    dw = np.asarray(depthwise_kernels, np.float32)
    pw = np.asarray(pointwise_kernels, np.float32)
    bs = np.asarray(biases, np.float32)
    in_maps = []
    for i in range(B):
        w1 = (dw[i].reshape(9 * C, C) @ pw[i, 0, 0]).reshape(3, 3, C, C)
        mean = xs[i].mean(axis=(0, 1))
        var = xs[i].var(axis=(0, 1))
        a = 1.0 / np.sqrt(var + EPS)
        # beta: constant (-a*mean) pushed through the conv, plus pw bias
        beta = np.einsum("c,ijco->o", -a * mean, w1) + bs[i]
        in_maps.append({
            "x": _pack_x(xs[i]),
            "u1": _wino_weights_folded(w1, rowscale=a),
            "u3": u3,
            "pb": np.ascontiguousarray(beta.reshape(C, 1)),
            "cb": cbr,
        })
    return in_maps


def _decode_out(o):
    # o [C, 2, 4, TPS] bf16 -> [H, W, C] f32
    a = np.asarray(o, np.float32).reshape(C, 2, 2, 2, THS, 32)
    # dims [c, s, a, b, th, tw] -> h = 32s+2th+a, w = 2tw+b
    return np.ascontiguousarray(
        a.transpose(1, 4, 2, 5, 3, 0).reshape(H, W, C))


def kernel(x, depthwise_kernels, pointwise_kernels, biases, conv_w, conv_b):
    B = 8
    nc = _get_nc()
    in_maps = make_in_maps(x, depthwise_kernels, pointwise_kernels, biases,
                           conv_w, conv_b)
    res = run_bass_kernel_spmd(nc, in_maps, core_ids=list(range(B)))
    return np.stack([_decode_out(res.results[i]["out"]) for i in range(B)])


# revision 4
# speedup vs baseline: 1.0112x; 1.0047x over previous
"""Trainium2 Bass kernel for nn_AdaConv — Winograd F(2x2,3x3), v3.

Per sample: instance-norm -> per-sample 3x3 conv (512->512, reflect pad)
-> per-sample 1x1 conv + bias -> shared 3x3 conv + bias (reflect pad).

All input-dependent preprocessing happens on the host:
- pointwise composed into the 3x3 conv (W1' = dw @ pw)
- Winograd weight transform with the F(2,3) half-factors folded in
- instance-norm: per-channel a = rsqrt(var+eps) folded into u1's ci rows;
  the mean offset and pointwise bias ship as a per-sample beta vector
  added at transform point (1,1) during the PSUM drain
- x ships in the kernel's C-major, H-reflect-padded (66 rows),
  W-parity-split bf16 layout

Device: two back-to-back Winograd convs.  PE does 1024 [128x128]x[128,512]
bf16 matmuls; scalar drains PSUM (plus the strided y2 commit scatter);
vector does all transforms (W-build, H-pass, inverse stage1/stage2);
gpsimd only issues DMAs and the tiny y2 border copies.  Output leaves
C-major bf16 and is decoded on the host.
"""

import numpy as np

from concourse.bass_utils import run_bass_kernel_spmd

import concourse.bass as bass
import concourse.mybir as mybir
import concourse.tile as tile

F32 = mybir.dt.float32
BF16 = mybir.dt.bfloat16
AF = mybir.ActivationFunctionType

H = W = 64
C = 512
NK = 4
EPS = 1e-5
NSTRIP = 2          # tile-row strips of 16 (32 image rows each)
THS = 16            # tile rows per strip
TPS = THS * 32      # tiles per strip (512)


def _hpass(nc, V, T, u, eng=None):
    """H-dim transform point u: T [128, NK, 34, 32] -> V [128, NK, THS, 32]."""
    e = eng or nc.vector
    if u == 0:
        e.tensor_sub(out=V, in0=T[:, :, 0:31:2, :], in1=T[:, :, 2:33:2, :])
    elif u == 1:
        e.tensor_add(out=V, in0=T[:, :, 1:32:2, :], in1=T[:, :, 2:33:2, :])
    elif u == 2:
        e.tensor_sub(out=V, in0=T[:, :, 2:33:2, :], in1=T[:, :, 1:32:2, :])
    else:
        e.tensor_sub(out=V, in0=T[:, :, 1:32:2, :], in1=T[:, :, 3:34:2, :])


def _wbuild(nc, T, src, w):
    """W-dim transform point w from parity-split rows src [128, NK, 34, 2, 32]
    into T [128, NK, 34, 32]."""
    p0 = src[:, :, :, 0, :]
    p1 = src[:, :, :, 1, :]
    if w == 0:
        nc.vector.memset(T[:, :, :, 0:1], 0.0)
        nc.vector.tensor_sub(out=T[:, :, :, 1:32],
                             in0=p1[:, :, :, 0:31], in1=p1[:, :, :, 1:32])
    elif w == 1:
        nc.vector.tensor_add(out=T, in0=p0, in1=p1)
    elif w == 2:
        nc.vector.tensor_sub(out=T, in0=p1, in1=p0)
    else:
        nc.vector.memset(T[:, :, :, 31:32], 0.0)
        nc.vector.tensor_sub(out=T[:, :, :, 0:31],
                             in0=p0[:, :, :, 0:31], in1=p0[:, :, :, 1:32])


def build():
    nc = bass.Bass()
    x_d = nc.declare_dram_parameter("x", [C, 66 * 64], BF16, isOutput=False)
    u1_d = nc.declare_dram_parameter("u1", [C, 16 * C], BF16, isOutput=False)
    u3_d = nc.declare_dram_parameter("u3", [C, 16 * C], BF16, isOutput=False)
    pb_d = nc.declare_dram_parameter("pb", [C, 1], F32, isOutput=False)
    cb_d = nc.declare_dram_parameter("cb", [C, 1], F32, isOutput=False)
    # out[c, s, ab, t]: h = 32s + 2*(t//32) + ab//2, w = 2*(t%32) + ab%2
    out_d = nc.declare_dram_parameter("out", [C, NSTRIP, 4, TPS], BF16,
                                      isOutput=True)

    with tile.TileContext(nc) as tc:
        with (
            tc.tile_pool(name="main", bufs=1) as mp,
            tc.tile_pool(name="psMM", bufs=8, space="PSUM") as psMM,
        ):
            pb_sb = mp.tile([128, NK, 1], F32, tag="pb")
            cb_sb = mp.tile([128, NK, 1], F32, tag="cb")

            y2b = mp.tile([128, NK, 66, 2, 32], BF16, tag="y2b")
            xb = mp.tile([128, NK, 66, 2, 32], BF16, tag="xb")
            # x: strip-1 rows (32..65) first -- they are processed first
            dmae = [nc.sync, nc.scalar, nc.gpsimd]
            for k in range(3):
                dmae[k].dma_start(out=xb[:, k, 32:66, :, :],
                                  in_=x_d[k * 128:(k + 1) * 128,
                                          32 * 64:66 * 64])
            for qi, (ra, rb) in enumerate(((32, 44), (44, 55), (55, 66))):
                dmae[qi].dma_start(out=xb[:, 3, ra:rb, :, :],
                                   in_=x_d[3 * 128:4 * 128,
                                           ra * 64:rb * 64])
            # interleave-order schedule: conv1 strips [1,0], conv3 strips [1,0]
            sched = [(1, s, w) for s in (1, 0) for w in range(4)] + \
                    [(3, s, w) for s in (1, 0) for w in range(4)]
            wsrc = {1: u1_d, 3: u3_d}

            def emit_chunk(i):
                cv, s, w = sched[i]
                Uw = mp.tile([128, NK, 4, C], BF16, tag="Uw", bufs=3,
                             name=f"Uw{cv}_{s}_{w}")
                for k in range(NK):
                    (nc.sync, nc.gpsimd)[(i + k) % 2].dma_start(
                        out=Uw[:, k, :, :],
                        in_=wsrc[cv][k * 128:(k + 1) * 128,
                                     w * 2048:(w + 1) * 2048])
                return Uw

            chunks = [emit_chunk(0), emit_chunk(1)]
            # x strip-0 rows after the first weight chunks (needed later)
            for k in range(NK):
                dmae[k % 3].dma_start(out=xb[:, k, 0:32, :, :],
                                      in_=x_d[k * 128:(k + 1) * 128, 0:32 * 64])
            for k in range(NK):
                nc.gpsimd.dma_start(out=pb_sb[:, k, :],
                                    in_=pb_d[k * 128:(k + 1) * 128, :])
                nc.gpsimd.dma_start(out=cb_sb[:, k, :],
                                    in_=cb_d[k * 128:(k + 1) * 128, :])

            def commit_y2(s, yst):
                # strided scatter into y2b rows on the scalar engine
                r0 = 1 + 32 * s
                for ab in (3, 2, 1, 0):
                    a, bb = ab // 2, ab % 2
                    nc.scalar.activation(
                        out=y2b[:, :, r0 + a:r0 + a + 31:2, bb, :],
                        in_=yst[:, :, ab, :], func=AF.Copy)
                # reflect border rows as soon as their source row exists
                if s == 1:
                    nc.scalar.activation(out=y2b[:, :, 65, :, :],
                                         in_=y2b[:, :, 63, :, :], func=AF.Copy)
                else:
                    nc.scalar.activation(out=y2b[:, :, 0, :, :],
                                         in_=y2b[:, :, 2, :, :], func=AF.Copy)

            def emit_out(s, yst):
                for k in range(NK):
                    (nc.sync, nc.gpsimd)[k % 2].dma_start(
                        out=out_d[k * 128:(k + 1) * 128, s, :, :],
                        in_=yst[:, k, :, :])

            def flush(pend):
                cv, s, w, Mq, yst = pend
                s0a = yst[:, :, 0, :]
                s1a = yst[:, :, 1, :]
                s2a = yst[:, :, 2, :]
                s3a = yst[:, :, 3, :]
                y0 = yst[:, :, 0:3:2, :]
                y1 = yst[:, :, 1:4:2, :]
                if w == 0:
                    # stage1 writes y0 slots (a0->slot0, a1->slot2) directly
                    nc.vector.tensor_add(out=s0a, in0=Mq[:, :, 0, :], in1=Mq[:, :, 1, :])
                    nc.vector.tensor_add(out=s0a, in0=s0a, in1=Mq[:, :, 2, :])
                    nc.vector.tensor_sub(out=s2a, in0=Mq[:, :, 1, :], in1=Mq[:, :, 2, :])
                    nc.vector.tensor_sub(out=s2a, in0=s2a, in1=Mq[:, :, 3, :])
                elif w == 1:
                    # stage1 into y1 slots, then y0 += y1
                    nc.vector.tensor_add(out=s1a, in0=Mq[:, :, 0, :], in1=Mq[:, :, 1, :])
                    nc.vector.tensor_add(out=s1a, in0=s1a, in1=Mq[:, :, 2, :])
                    nc.vector.tensor_sub(out=s3a, in0=Mq[:, :, 1, :], in1=Mq[:, :, 2, :])
                    nc.vector.tensor_sub(out=s3a, in0=s3a, in1=Mq[:, :, 3, :])
                    nc.vector.tensor_add(out=y0, in0=y0, in1=y1)
                else:
                    tq = mp.tile([128, NK, 2, TPS], BF16, tag="tq", bufs=1,
                                 name=f"tq{cv}_{s}_{w}")
                    t0 = tq[:, :, 0, :]
                    t1 = tq[:, :, 1, :]
                    # t1 first: at w==3 it alone gates the slot-3 commit
                    nc.vector.tensor_sub(out=t1, in0=Mq[:, :, 1, :], in1=Mq[:, :, 2, :])
                    nc.vector.tensor_sub(out=t1, in0=t1, in1=Mq[:, :, 3, :])
                    boundary = (cv == 1 and s == 0 and w == 3)
                    if not boundary:
                        nc.vector.tensor_add(out=t0, in0=Mq[:, :, 0, :], in1=Mq[:, :, 1, :])
                        nc.vector.tensor_add(out=t0, in0=t0, in1=Mq[:, :, 2, :])
                    t = tq[:, :, :, :]
                    if w == 2:
                        nc.vector.tensor_add(out=y0, in0=y0, in1=t)
                        nc.vector.tensor_sub(out=y1, in0=y1, in1=t)
                    elif cv == 1:
                        nc.vector.tensor_sub(out=y1[:, :, 1, :],
                                             in0=y1[:, :, 1, :],
                                             in1=t[:, :, 1, :])
                        if boundary:
                            # tiny row-32 commits unblock conv3-strip1 early
                            for bb in (1, 0):
                                nc.scalar.activation(
                                    out=y2b[:, :, 32, bb, :],
                                    in_=yst[:, :, 2 + bb, 480:512],
                                    func=AF.Copy)

                            def _rest():
                                nc.vector.tensor_add(out=t0, in0=Mq[:, :, 0, :],
                                                     in1=Mq[:, :, 1, :])
                                nc.vector.tensor_add(out=t0, in0=t0,
                                                     in1=Mq[:, :, 2, :])
                                nc.vector.tensor_sub(out=y1[:, :, 0, :],
                                                     in0=y1[:, :, 0, :],
                                                     in1=t[:, :, 0, :])
                                commit_y2(s, yst)
                            deferred[0] = _rest
                        else:
                            nc.vector.tensor_sub(out=y1[:, :, 0, :],
                                                 in0=y1[:, :, 0, :],
                                                 in1=t[:, :, 0, :])
                            commit_y2(s, yst)
                    else:
                        # per-k: finish y1 and ship that k immediately
                        for k in range(NK):
                            nc.vector.tensor_sub(out=y1[:, k, :, :],
                                                 in0=y1[:, k, :, :],
                                                 in1=t[:, k, :, :])
                            (nc.sync, nc.scalar)[k % 2].dma_start(
                                out=out_d[k * 128:(k + 1) * 128, s, :, :],
                                in_=yst[:, k, :, :])

            deferred = [None]
            srcs = {1: xb, 3: y2b}
            bias5 = {1: pb_sb, 3: cb_sb}
            pend = None
            yst_by = {}
            for i, (cv, s, w) in enumerate(sched):
                if i + 2 < len(sched):
                    chunks.append(emit_chunk(i + 2))
                Uw = chunks[i]
                src = srcs[cv]
                r0 = 32 * s
                if w == 0:
                    yst_by[(cv, s)] = mp.tile([128, NK, 4, TPS], BF16,
                                              tag="yst", bufs=2,
                                              name=f"yst{cv}_{s}")
                yst = yst_by[(cv, s)]
                if (cv, s, w) == (3, 1, 0):
                    T = preT  # big part prebuilt before the boundary flush
                    pp1 = src[:, :, 32:33, 1, :]
                    nc.vector.tensor_sub(out=T[:, :, 0:1, 1:32],
                                         in0=pp1[:, :, :, 0:31],
                                         in1=pp1[:, :, :, 1:32])
                else:
                    T = mp.tile([128, NK, 34, 32], BF16, tag="T", bufs=1,
                                name=f"T{cv}_{s}_{w}")
                    _wbuild(nc, T, src[:, :, r0:r0 + 34, :, :], w)
                Mq = mp.tile([128, NK, 4, TPS], BF16, tag="Mq", bufs=2,
                             name=f"Mq{cv}_{s}_{w}")
                for u in range(4):
                    ptid = w * 4 + u
                    V = mp.tile([128, NK, THS, 32], BF16, tag="V", bufs=3,
                                name=f"V{cv}_{s}_{ptid}")
                    _hpass(nc, V, T, u)
                    if deferred[0] is not None and (cv, s, u) == (3, 1, 0):
                        deferred[0]()
                        deferred[0] = None
                    for co in range(NK):
                        ps = psMM.tile([128, TPS], F32, tag="mm",
                                       name=f"ps{cv}_{s}_{ptid}_{co}")
                        for ci in range(NK):
                            nc.tensor.matmul(
                                ps[:, :],
                                Uw[:, ci, u, co * 128:(co + 1) * 128],
                                V[:, ci, :, :],
                                start=(ci == 0), stop=(ci == NK - 1))
                        if ptid == 5:
                            nc.scalar.activation(
                                out=Mq[:, co, u, :], in_=ps,
                                func=AF.Identity, bias=bias5[cv][:, co, :])
                        else:
                            nc.scalar.activation(
                                out=Mq[:, co, u, :], in_=ps, func=AF.Copy)
                if pend is not None:
                    flush(pend)
                pend = (cv, s, w, Mq, yst)
                if cv == 1 and s == 0 and w == 3:
                    # prebuild conv3-s1-w0 T rows 1..33 (y2b rows 33..65, ready)
                    preT = mp.tile([128, NK, 34, 32], BF16, tag="T", bufs=1,
                                   name="Tpre")
                    nc.vector.memset(preT[:, :, :, 0:1], 0.0)
                    p1b = y2b[:, :, 33:66, 1, :]
                    nc.vector.tensor_sub(out=preT[:, :, 1:34, 1:32],
                                         in0=p1b[:, :, :, 0:31],
                                         in1=p1b[:, :, :, 1:32])
                    # conv boundary: eager flush (conv3 strip1 needs row 32)
                    flush(pend)
                    pend = None
            if pend is not None:
                flush(pend)

    return nc


# ---- walrus single-wait workaround (same as baseline) ----
import concourse.tile as tile_mod
from concourse.vector_clock import ScopedClock

MAX_WAITS = 1
_counter = [0]


def _fresh_name(base):
    _counter[0] += 1
    return f"{base}-wsplit-{_counter[0]}"


_orig_add_instruction = tile_mod.TileContext._add_instruction


def _patched_add_instruction(self, inst):
    si = getattr(inst, "sync_info", None)
    if si is not None and si.on_wait is not None and len(si.on_wait) > MAX_WAITS:
        waits = list(si.on_wait)
        for w in waits[:-MAX_WAITS]:
            nop = mybir.InstNoOp(
                name=_fresh_name(inst.name),
                sync_info=mybir.SyncInfo(on_wait=[w], on_update=[]),
                bass_nofuse=True,
                engine=inst.engine,
            )
            _orig_add_instruction(self, nop)
        inst.sync_info = mybir.SyncInfo(
            on_wait=waits[-MAX_WAITS:], on_update=list(si.on_update)
        )
    _orig_add_instruction(self, inst)


def _split_tail_waits(nc, raw):
    si = raw.sync_info
    waits = list(si.on_wait) if si is not None else []
    if len(waits) <= MAX_WAITS:
        return
    updates = list(si.on_update) if si is not None else []
    raw.sync_info = mybir.SyncInfo(on_wait=waits[:MAX_WAITS], on_update=updates)
    for i in range(MAX_WAITS, len(waits), MAX_WAITS):
        extra = nc.sync.drain()
        extra.ins.sync_info = mybir.SyncInfo(
            on_wait=waits[i : i + MAX_WAITS], on_update=[]
        )


def _patched_drain_and_barrier(self, tick_clock, wait_clock):
    nc = self.nc
    drain_inst = nc.sync.drain()
    wait_clock.add_sem_waits(
        drain_inst.ins, ScopedClock({None: tick_clock.global_clock})
    )
    _split_tail_waits(nc, drain_inst.ins)

    nc.all_engine_barrier()
    assert self.sems is not None
    popped = nc._tile_sem_poison_stack.pop()
    assert popped is self._sem_poison
    nc.clear_and_free_semaphores(list(self.sems.allocated().values()))
    nc.all_engine_barrier()


def install():
    tile_mod.TileContext._add_instruction = _patched_add_instruction
    tile_mod.TileContext._drain_and_barrier = _patched_drain_and_barrier


_cached_nc = None


def _get_nc():
    global _cached_nc
    if _cached_nc is None:
        install()
        _cached_nc = build()
    return _cached_nc


def _wino_weights_folded(w, rowscale=None):
    """Host F(2,3) weight transform with the inverse-transform half-factors
    folded in (standard G with 1/2 rows): w [3,3,C,Co] -> [C, 16*Co] bf16.
    rowscale [C]: optional per-input-channel scale (instance-norm a)."""
    import ml_dtypes
    G = np.array([[1, 0, 0], [.5, .5, .5], [.5, -.5, .5], [0, 0, 1]], np.float32)
    U = np.einsum("ui,ijco,vj->uvco", G, np.asarray(w, np.float32), G,
                  optimize=True)
    if rowscale is not None:
        U = U * rowscale[None, None, :, None]
    U = U.transpose(2, 1, 0, 3).reshape(w.shape[2], 16 * w.shape[3])
    return np.ascontiguousarray(U.astype(ml_dtypes.bfloat16))


def _pack_x(xi):
    """x [H,W,C] f32 -> [C, 66*64] bf16 in C-major, H-reflect-padded,
    W-parity-split layout: out[c, r, p, j] = x[r-1, 2j+p, c]."""
    import ml_dtypes
    xc = np.asarray(xi, np.float32).transpose(2, 0, 1)          # [C,H,W]
    core = xc.reshape(C, H, 32, 2).transpose(0, 1, 3, 2)        # [C,H,p,j]
    arr = np.empty((C, 66, 2, 32), np.float32)
    arr[:, 1:65] = core
    arr[:, 0] = core[:, 1]
    arr[:, 65] = core[:, 62]
    return np.ascontiguousarray(
        arr.reshape(C, 66 * 64).astype(ml_dtypes.bfloat16))


def make_in_maps(x, depthwise_kernels, pointwise_kernels, biases, conv_w, conv_b):
    B = 8
    u3 = _wino_weights_folded(np.asarray(conv_w, np.float32))
    cbr = np.ascontiguousarray(np.asarray(conv_b, np.float32).reshape(C, 1))
    xs = np.asarray(x, np.float32)
    dw = np.asarray(depthwise_kernels, np.float32)
    pw = np.asarray(pointwise_kernels, np.float32)
    bs = np.asarray(biases, np.float32)
    in_maps = []
    for i in range(B):
        w1 = (dw[i].reshape(9 * C, C) @ pw[i, 0, 0]).reshape(3, 3, C, C)
        mean = xs[i].mean(axis=(0, 1))
        var = xs[i].var(axis=(0, 1))
        a = 1.0 / np.sqrt(var + EPS)
        # beta: constant (-a*mean) pushed through the conv, plus pw bias
        beta = np.einsum("c,ijco->o", -a * mean, w1) + bs[i]
        in_maps.append({
            "x": _pack_x(xs[i]),
            "u1": _wino_weights_folded(w1, rowscale=a),
            "u3": u3,
            "pb": np.ascontiguousarray(beta.reshape(C, 1)),
            "cb": cbr,
        })
    return in_maps


def _decode_out(o):
    # o [C, 2, 4, TPS] bf16 -> [H, W, C] f32
    a = np.asarray(o, np.float32).reshape(C, 2, 2, 2, THS, 32)
    # dims [c, s, a, b, th, tw] -> h = 32s+2th+a, w = 2tw+b
    return np.ascontiguousarray(
        a.transpose(1, 4, 2, 5, 3, 0).reshape(H, W, C))


def kernel(x, depthwise_kernels, pointwise_kernels, biases, conv_w, conv_b):
    B = 8
    nc = _get_nc()
    in_maps = make_in_maps(x, depthwise_kernels, pointwise_kernels, biases,
                           conv_w, conv_b)
    res = run_bass_kernel_spmd(nc, in_maps, core_ids=list(range(B)))
    return np.stack([_decode_out(res.results[i]["out"]) for i in range(B)])


# revision 6
# speedup vs baseline: 1.0380x; 1.0265x over previous
"""Trainium2 Bass kernel for nn_AdaConv — Winograd F(2x2,3x3), v3.

Per sample: instance-norm -> per-sample 3x3 conv (512->512, reflect pad)
-> per-sample 1x1 conv + bias -> shared 3x3 conv + bias (reflect pad).

All input-dependent preprocessing happens on the host:
- pointwise composed into the 3x3 conv (W1' = dw @ pw)
- Winograd weight transform with the F(2,3) half-factors folded in
- instance-norm: per-channel a = rsqrt(var+eps) folded into u1's ci rows;
  the mean offset and pointwise bias ship as a per-sample beta vector
  added at transform point (1,1) during the PSUM drain
- x ships in the kernel's C-major, H-reflect-padded (66 rows),
  W-parity-split bf16 layout

Device: two back-to-back Winograd convs.  PE does 1024 [128x128]x[128,512]
bf16 matmuls; scalar drains PSUM (plus the strided y2 commit scatter);
vector does all transforms (W-build, H-pass, inverse stage1/stage2);
gpsimd only issues DMAs and the tiny y2 border copies.  Output leaves
C-major bf16 and is decoded on the host.
"""

import numpy as np

from concourse.bass_utils import run_bass_kernel_spmd

import concourse.bass as bass
import concourse.mybir as mybir
import concourse.tile as tile

F32 = mybir.dt.float32
BF16 = mybir.dt.bfloat16
AF = mybir.ActivationFunctionType

H = W = 64
C = 512
NK = 4
EPS = 1e-5
NSTRIP = 2          # tile-row strips of 16 (32 image rows each)
THS = 16            # tile rows per strip
TPS = THS * 32      # tiles per strip (512)


def _hpass(nc, V, T, u, eng=None):
    """H-dim transform point u: T [128, NK, 34, 32] -> V [128, NK, THS, 32]."""
    e = eng or nc.vector
    if u == 0:
        e.tensor_sub(out=V, in0=T[:, :, 0:31:2, :], in1=T[:, :, 2:33:2, :])
    elif u == 1:
        e.tensor_add(out=V, in0=T[:, :, 1:32:2, :], in1=T[:, :, 2:33:2, :])
    elif u == 2:
        e.tensor_sub(out=V, in0=T[:, :, 2:33:2, :], in1=T[:, :, 1:32:2, :])
    else:
        e.tensor_sub(out=V, in0=T[:, :, 1:32:2, :], in1=T[:, :, 3:34:2, :])


def _wbuild(nc, T, src, w):
    """W-dim transform point w from parity-split rows src [128, NK, 34, 2, 32]
    into T [128, NK, 34, 32]."""
    p0 = src[:, :, :, 0, :]
    p1 = src[:, :, :, 1, :]
    if w == 0:
        nc.vector.memset(T[:, :, :, 0:1], 0.0)
        nc.vector.tensor_sub(out=T[:, :, :, 1:32],
                             in0=p1[:, :, :, 0:31], in1=p1[:, :, :, 1:32])
    elif w == 1:
        nc.vector.tensor_add(out=T, in0=p0, in1=p1)
    elif w == 2:
        nc.vector.tensor_sub(out=T, in0=p1, in1=p0)
    else:
        nc.vector.memset(T[:, :, :, 31:32], 0.0)
        nc.vector.tensor_sub(out=T[:, :, :, 0:31],
                             in0=p0[:, :, :, 0:31], in1=p0[:, :, :, 1:32])


def build():
    nc = bass.Bass()
    x_d = nc.declare_dram_parameter("x", [C, 66 * 64], BF16, isOutput=False)
    u1_d = nc.declare_dram_parameter("u1", [C, 16 * C], BF16, isOutput=False)
    u3_d = nc.declare_dram_parameter("u3", [C, 16 * C], BF16, isOutput=False)
    pb_d = nc.declare_dram_parameter("pb", [C, 1], F32, isOutput=False)
    cb_d = nc.declare_dram_parameter("cb", [C, 1], F32, isOutput=False)
    # out[c, s, ab, t]: h = 32s + 2*(t//32) + ab//2, w = 2*(t%32) + ab%2
    out_d = nc.declare_dram_parameter("out", [C, NSTRIP, 4, TPS], BF16,
                                      isOutput=True)

    with tile.TileContext(nc) as tc:
        with (
            tc.tile_pool(name="main", bufs=1) as mp,
            tc.tile_pool(name="psMM", bufs=8, space="PSUM") as psMM,
        ):
            pb_sb = mp.tile([128, NK, 1], F32, tag="pb")
            cb_sb = mp.tile([128, NK, 1], F32, tag="cb")

            y2b = mp.tile([128, NK, 66, 2, 32], BF16, tag="y2b")
            xb = mp.tile([128, NK, 66, 2, 32], BF16, tag="xb")
            # PE warm-up: throwaway matmuls on (uninitialized) y2b while the
            # input DMAs are in flight -- ramps the PE clock to max p-state
            # before the first real chain. Results are never read; y2b's
            # first real writer (the commits) comes long after.
            for i in range(55):
                wps = psMM.tile([128, TPS], F32, tag="mm", name=f"warm{i}")
                nc.tensor.matmul(wps[:, :], y2b[:, 0, 0:2, :, :],
                                 y2b[:, 0, 2:10, :, :], start=True, stop=True)
            # x: strip-1 rows (32..65) first -- they are processed first
            dmae = [nc.sync, nc.scalar, nc.gpsimd]
            for k in range(3):
                dmae[k].dma_start(out=xb[:, k, 32:66, :, :],
                                  in_=x_d[k * 128:(k + 1) * 128,
                                          32 * 64:66 * 64])
            for qi, (ra, rb) in enumerate(((32, 44), (44, 55), (55, 66))):
                dmae[qi].dma_start(out=xb[:, 3, ra:rb, :, :],
                                   in_=x_d[3 * 128:4 * 128,
                                           ra * 64:rb * 64])
            # interleave-order schedule: conv1 strips [1,0], conv3 strips [1,0]
            sched = [(1, s, w) for s in (1, 0) for w in range(4)] + \
                    [(3, s, w) for s in (1, 0) for w in range(4)]
            wsrc = {1: u1_d, 3: u3_d}

            def emit_chunk(i):
                cv, s, w = sched[i]
                Uw = mp.tile([128, NK, 4, C], BF16, tag="Uw", bufs=3,
                             name=f"Uw{cv}_{s}_{w}")
                for k in range(NK):
                    (nc.sync, nc.gpsimd)[(i + k) % 2].dma_start(
                        out=Uw[:, k, :, :],
                        in_=wsrc[cv][k * 128:(k + 1) * 128,
                                     w * 2048:(w + 1) * 2048])
                return Uw

            chunks = [emit_chunk(0), emit_chunk(1)]
            # x strip-0 rows after the first weight chunks (needed later)
            for k in range(NK):
                dmae[k % 3].dma_start(out=xb[:, k, 0:32, :, :],
                                      in_=x_d[k * 128:(k + 1) * 128, 0:32 * 64])
            for k in range(NK):
                nc.gpsimd.dma_start(out=pb_sb[:, k, :],
                                    in_=pb_d[k * 128:(k + 1) * 128, :])
                nc.gpsimd.dma_start(out=cb_sb[:, k, :],
                                    in_=cb_d[k * 128:(k + 1) * 128, :])

            def commit_y2(s, yst):
                # strided scatter into y2b rows on the scalar engine
                r0 = 1 + 32 * s
                for ab in (3, 2, 1, 0):
                    a, bb = ab // 2, ab % 2
                    nc.scalar.activation(
                        out=y2b[:, :, r0 + a:r0 + a + 31:2, bb, :],
                        in_=yst[:, :, ab, :], func=AF.Copy)
                # reflect border rows as soon as their source row exists
                if s == 1:
                    nc.scalar.activation(out=y2b[:, :, 65, :, :],
                                         in_=y2b[:, :, 63, :, :], func=AF.Copy)
                else:
                    nc.scalar.activation(out=y2b[:, :, 0, :, :],
                                         in_=y2b[:, :, 2, :, :], func=AF.Copy)

            def emit_out(s, yst):
                for k in range(NK):
                    (nc.sync, nc.gpsimd)[k % 2].dma_start(
                        out=out_d[k * 128:(k + 1) * 128, s, :, :],
                        in_=yst[:, k, :, :])

            def flush(pend):
                cv, s, w, Mq, yst = pend
                s0a = yst[:, :, 0, :]
                s1a = yst[:, :, 1, :]
                s2a = yst[:, :, 2, :]
                s3a = yst[:, :, 3, :]
                y0 = yst[:, :, 0:3:2, :]
                y1 = yst[:, :, 1:4:2, :]
                if w == 0:
                    # stage1 writes y0 slots (a0->slot0, a1->slot2) directly
                    nc.vector.tensor_add(out=s0a, in0=Mq[:, :, 0, :], in1=Mq[:, :, 1, :])
                    nc.vector.tensor_add(out=s0a, in0=s0a, in1=Mq[:, :, 2, :])
                    nc.vector.tensor_sub(out=s2a, in0=Mq[:, :, 1, :], in1=Mq[:, :, 2, :])
                    nc.vector.tensor_sub(out=s2a, in0=s2a, in1=Mq[:, :, 3, :])
                elif w == 1:
                    # stage1 into y1 slots, then y0 += y1
                    nc.vector.tensor_add(out=s1a, in0=Mq[:, :, 0, :], in1=Mq[:, :, 1, :])
                    nc.vector.tensor_add(out=s1a, in0=s1a, in1=Mq[:, :, 2, :])
                    nc.vector.tensor_sub(out=s3a, in0=Mq[:, :, 1, :], in1=Mq[:, :, 2, :])
                    nc.vector.tensor_sub(out=s3a, in0=s3a, in1=Mq[:, :, 3, :])
                    nc.vector.tensor_add(out=y0, in0=y0, in1=y1)
                else:
                    tq = mp.tile([128, NK, 2, TPS], BF16, tag="tq", bufs=1,
                                 name=f"tq{cv}_{s}_{w}")
                    t0 = tq[:, :, 0, :]
                    t1 = tq[:, :, 1, :]
                    boundary = (cv == 1 and s == 0 and w == 3)
                    if boundary:
                        # th=15 slice only: the single row conv3-s1 waits on
                        nc.vector.tensor_sub(out=t1[:, :, 480:512],
                                             in0=Mq[:, :, 1, 480:512],
                                             in1=Mq[:, :, 2, 480:512])
                        nc.vector.tensor_sub(out=t1[:, :, 480:512],
                                             in0=t1[:, :, 480:512],
                                             in1=Mq[:, :, 3, 480:512])
                    else:
                        nc.vector.tensor_sub(out=t1, in0=Mq[:, :, 1, :], in1=Mq[:, :, 2, :])
                        nc.vector.tensor_sub(out=t1, in0=t1, in1=Mq[:, :, 3, :])
                    if not boundary:
                        nc.vector.tensor_add(out=t0, in0=Mq[:, :, 0, :], in1=Mq[:, :, 1, :])
                        nc.vector.tensor_add(out=t0, in0=t0, in1=Mq[:, :, 2, :])
                    t = tq[:, :, :, :]
                    if w == 2:
                        nc.vector.tensor_add(out=y0, in0=y0, in1=t)
                        nc.vector.tensor_sub(out=y1, in0=y1, in1=t)
                    elif cv == 1:
                        if boundary:
                            nc.vector.tensor_sub(out=y1[:, :, 1, 480:512],
                                                 in0=y1[:, :, 1, 480:512],
                                                 in1=t[:, :, 1, 480:512])
                        else:
                            nc.vector.tensor_sub(out=y1[:, :, 1, :],
                                                 in0=y1[:, :, 1, :],
                                                 in1=t[:, :, 1, :])
                        if boundary:
                            # tiny row-32 commits unblock conv3-strip1 early
                            for bb in (1, 0):
                                nc.scalar.activation(
                                    out=y2b[:, :, 32, bb, :],
                                    in_=yst[:, :, 2 + bb, 480:512],
                                    func=AF.Copy)

                            def _rest():
                                nc.vector.tensor_sub(out=t1[:, :, 0:480],
                                                     in0=Mq[:, :, 1, 0:480],
                                                     in1=Mq[:, :, 2, 0:480])
                                nc.vector.tensor_sub(out=t1[:, :, 0:480],
                                                     in0=t1[:, :, 0:480],
                                                     in1=Mq[:, :, 3, 0:480])
                                nc.vector.tensor_sub(out=y1[:, :, 1, 0:480],
                                                     in0=y1[:, :, 1, 0:480],
                                                     in1=t[:, :, 1, 0:480])
                                nc.vector.tensor_add(out=t0, in0=Mq[:, :, 0, :],
                                                     in1=Mq[:, :, 1, :])
                                nc.vector.tensor_add(out=t0, in0=t0,
                                                     in1=Mq[:, :, 2, :])
                                nc.vector.tensor_sub(out=y1[:, :, 0, :],
                                                     in0=y1[:, :, 0, :],
                                                     in1=t[:, :, 0, :])
                                commit_y2(s, yst)
                            deferred[0] = _rest
                        else:
                            nc.vector.tensor_sub(out=y1[:, :, 0, :],
                                                 in0=y1[:, :, 0, :],
                                                 in1=t[:, :, 0, :])
                            commit_y2(s, yst)
                    else:
                        # per-k: finish y1 and ship that k immediately
                        for k in range(NK):
                            nc.vector.tensor_sub(out=y1[:, k, :, :],
                                                 in0=y1[:, k, :, :],
                                                 in1=t[:, k, :, :])
                            (nc.sync, nc.scalar)[k % 2].dma_start(
                                out=out_d[k * 128:(k + 1) * 128, s, :, :],
                                in_=yst[:, k, :, :])

            deferred = [None]
            srcs = {1: xb, 3: y2b}
            bias5 = {1: pb_sb, 3: cb_sb}
            pend = None
            yst_by = {}
            for i, (cv, s, w) in enumerate(sched):
                if i + 2 < len(sched):
                    chunks.append(emit_chunk(i + 2))
                Uw = chunks[i]
                src = srcs[cv]
                r0 = 32 * s
                if w == 0:
                    yst_by[(cv, s)] = mp.tile([128, NK, 4, TPS], BF16,
                                              tag="yst", bufs=2,
                                              name=f"yst{cv}_{s}")
                yst = yst_by[(cv, s)]
                if (cv, s, w) == (3, 1, 0):
                    T = preT  # big part prebuilt before the boundary flush
                    pp1 = src[:, :, 32:33, 1, :]
                    nc.vector.tensor_sub(out=T[:, :, 0:1, 1:32],
                                         in0=pp1[:, :, :, 0:31],
                                         in1=pp1[:, :, :, 1:32])
                else:
                    T = mp.tile([128, NK, 34, 32], BF16, tag="T", bufs=1,
                                name=f"T{cv}_{s}_{w}")
                    _wbuild(nc, T, src[:, :, r0:r0 + 34, :, :], w)
                Mq = mp.tile([128, NK, 4, TPS], BF16, tag="Mq", bufs=2,
                             name=f"Mq{cv}_{s}_{w}")
                for u in range(4):
                    ptid = w * 4 + u
                    V = mp.tile([128, NK, THS, 32], BF16, tag="V", bufs=3,
                                name=f"V{cv}_{s}_{ptid}")
                    _hpass(nc, V, T, u)
                    if deferred[0] is not None and (cv, s, u) == (3, 1, 0):
                        deferred[0]()
                        deferred[0] = None
                    for co in range(NK):
                        ps = psMM.tile([128, TPS], F32, tag="mm",
                                       name=f"ps{cv}_{s}_{ptid}_{co}")
                        for ci in range(NK):
                            nc.tensor.matmul(
                                ps[:, :],
                                Uw[:, ci, u, co * 128:(co + 1) * 128],
                                V[:, ci, :, :],
                                start=(ci == 0), stop=(ci == NK - 1))
                        if ptid == 5:
                            nc.scalar.activation(
                                out=Mq[:, co, u, :], in_=ps,
                                func=AF.Identity, bias=bias5[cv][:, co, :])
                        else:
                            nc.scalar.activation(
                                out=Mq[:, co, u, :], in_=ps, func=AF.Copy)
                if pend is not None:
                    flush(pend)
                pend = (cv, s, w, Mq, yst)
                if cv == 1 and s == 0 and w == 3:
                    # keep the PE clock hot through the boundary gap (xb is dead)
                    for i in range(20):
                        wps = psMM.tile([128, TPS], F32, tag="mm",
                                        name=f"bwarm{i}")
                        nc.tensor.matmul(wps[:, :], xb[:, 0, 0:2, :, :],
                                         xb[:, 0, 2:10, :, :],
                                         start=True, stop=True)
                    # prebuild conv3-s1-w0 T rows 1..33 (y2b rows 33..65, ready)
                    preT = mp.tile([128, NK, 34, 32], BF16, tag="T", bufs=1,
                                   name="Tpre")
                    nc.vector.memset(preT[:, :, :, 0:1], 0.0)
                    p1b = y2b[:, :, 33:66, 1, :]
                    nc.vector.tensor_sub(out=preT[:, :, 1:34, 1:32],
                                         in0=p1b[:, :, :, 0:31],
                                         in1=p1b[:, :, :, 1:32])
                    # conv boundary: eager flush (conv3 strip1 needs row 32)
                    flush(pend)
                    pend = None
            if pend is not None:
                flush(pend)

    return nc


# ---- walrus single-wait workaround (same as baseline) ----
import concourse.tile as tile_mod
from concourse.vector_clock import ScopedClock

MAX_WAITS = 1
_counter = [0]


def _fresh_name(base):
    _counter[0] += 1
    return f"{base}-wsplit-{_counter[0]}"


_orig_add_instruction = tile_mod.TileContext._add_instruction


def _patched_add_instruction(self, inst):
    si = getattr(inst, "sync_info", None)
    if si is not None and si.on_wait is not None and len(si.on_wait) > MAX_WAITS:
        waits = list(si.on_wait)
        for w in waits[:-MAX_WAITS]:
            nop = mybir.InstNoOp(
                name=_fresh_name(inst.name),
                sync_info=mybir.SyncInfo(on_wait=[w], on_update=[]),
                bass_nofuse=True,
                engine=inst.engine,
            )
            _orig_add_instruction(self, nop)
        inst.sync_info = mybir.SyncInfo(
            on_wait=waits[-MAX_WAITS:], on_update=list(si.on_update)
        )
    _orig_add_instruction(self, inst)


def _split_tail_waits(nc, raw):
    si = raw.sync_info
    waits = list(si.on_wait) if si is not None else []
    if len(waits) <= MAX_WAITS:
        return
    updates = list(si.on_update) if si is not None else []
    raw.sync_info = mybir.SyncInfo(on_wait=waits[:MAX_WAITS], on_update=updates)
    for i in range(MAX_WAITS, len(waits), MAX_WAITS):
        extra = nc.sync.drain()
        extra.ins.sync_info = mybir.SyncInfo(
            on_wait=waits[i : i + MAX_WAITS], on_update=[]
        )


def _patched_drain_and_barrier(self, tick_clock, wait_clock):
    nc = self.nc
    drain_inst = nc.sync.drain()
    wait_clock.add_sem_waits(
        drain_inst.ins, ScopedClock({None: tick_clock.global_clock})
    )
    _split_tail_waits(nc, drain_inst.ins)

    nc.all_engine_barrier()
    assert self.sems is not None
    popped = nc._tile_sem_poison_stack.pop()
    assert popped is self._sem_poison
    nc.clear_and_free_semaphores(list(self.sems.allocated().values()))


def install():
    tile_mod.TileContext._add_instruction = _patched_add_instruction
    tile_mod.TileContext._drain_and_barrier = _patched_drain_and_barrier


_cached_nc = None


def _get_nc():
    global _cached_nc
    if _cached_nc is None:
        install()
        _cached_nc = build()
    return _cached_nc


def _wino_weights_folded(w, rowscale=None):
    """Host F(2,3) weight transform with the inverse-transform half-factors
    folded in (standard G with 1/2 rows): w [3,3,C,Co] -> [C, 16*Co] bf16.
    rowscale [C]: optional per-input-channel scale (instance-norm a)."""
    import ml_dtypes
    G = np.array([[1, 0, 0], [.5, .5, .5], [.5, -.5, .5], [0, 0, 1]], np.float32)
    U = np.einsum("ui,ijco,vj->uvco", G, np.asarray(w, np.float32), G,
                  optimize=True)
    if rowscale is not None:
        U = U * rowscale[None, None, :, None]
    U = U.transpose(2, 1, 0, 3).reshape(w.shape[2], 16 * w.shape[3])
    return np.ascontiguousarray(U.astype(ml_dtypes.bfloat16))


def _pack_x(xi):
    """x [H,W,C] f32 -> [C, 66*64] bf16 in C-major, H-reflect-padded,
    W-parity-split layout: out[c, r, p, j] = x[r-1, 2j+p, c]."""
    import ml_dtypes
    xc = np.asarray(xi, np.float32).transpose(2, 0, 1)          # [C,H,W]
    core = xc.reshape(C, H, 32, 2).transpose(0, 1, 3, 2)        # [C,H,p,j]
    arr = np.empty((C, 66, 2, 32), np.float32)
    arr[:, 1:65] = core
    arr[:, 0] = core[:, 1]
    arr[:, 65] = core[:, 62]
    return np.ascontiguousarray(
        arr.reshape(C, 66 * 64).astype(ml_dtypes.bfloat16))


def make_in_maps(x, depthwise_kernels, pointwise_kernels, biases, conv_w, conv_b):
    B = 8
    u3 = _wino_weights_folded(np.asarray(conv_w, np.float32))
    cbr = np.ascontiguousarray(np.asarray(conv_b, np.float32).reshape(C, 1))
    xs = np.asarray(x, np.float32)
    dw = np.asarray(depthwise_kernels, np.float32)
    pw = np.asarray(pointwise_kernels, np.float32)
    bs = np.asarray(biases, np.float32)
    in_maps = []
    for i in range(B):
        w1 = (dw[i].reshape(9 * C, C) @ pw[i, 0, 0]).reshape(3, 3, C, C)
        mean = xs[i].mean(axis=(0, 1))
        var = xs[i].var(axis=(0, 1))
        a = 1.0 / np.sqrt(var + EPS)
        # beta: constant (-a*mean) pushed through the conv, plus pw bias
        beta = np.einsum("c,ijco->o", -a * mean, w1) + bs[i]
        in_maps.append({
            "x": _pack_x(xs[i]),
            "u1": _wino_weights_folded(w1, rowscale=a),
            "u3": u3,
            "pb": np.ascontiguousarray(beta.reshape(C, 1)),
            "cb": cbr,
        })
    return in_maps


def _decode_out(o):
    # o [C, 2, 4, TPS] bf16 -> [H, W, C] f32
    a = np.asarray(o, np.float32).reshape(C, 2, 2, 2, THS, 32)
    # dims [c, s, a, b, th, tw] -> h = 32s+2th+a, w = 2tw+b
    return np.ascontiguousarray(
        a.transpose(1, 4, 2, 5, 3, 0).reshape(H, W, C))


def kernel(x, depthwise_kernels, pointwise_kernels, biases, conv_w, conv_b):
    B = 8
    nc = _get_nc()
    in_maps = make_in_maps(x, depthwise_kernels, pointwise_kernels, biases,
                           conv_w, conv_b)
    res = run_bass_kernel_spmd(nc, in_maps, core_ids=list(range(B)))
    return np.stack([_decode_out(res.results[i]["out"]) for i in range(B)])
